# revision 31
# baseline (speedup 1.0000x reference)
"""DeepSets segment-reduce kernel for 8x Trainium2 NeuronCores.

Strategy (all shapes hardcoded for N=500000, C=H=128, O=64, NSEG=2048):
  - Transposed activation layout: features on SBUF partitions, nodes on the
    free axis, so segment reductions are free-axis reduces.
  - Whole-segment sharding: every segment is assigned entirely to one core,
    round-robin by global sorted-width rank.  All 8 cores then share an
    identical compile-time slot/tile geometry (SPMD-safe); per-core padding
    is <1%.  No collective is needed - the host gather is the unshard.
  - Encoder BN is folded into the linear weights (W' = W * g*rsqrt(v+eps),
    b' = (b-m)*g*rsqrt(v+eps) + beta), so each layer is relu(W'x + b').
  - All encoder matmul operands are bf16: the PE streams bf16 moving rows
    at 1 cycle/row vs ~2 for fp32/f32r, and the x DMA halves.  PSUM
    accumulation stays fp32; rel-err vs the fp32 reference is ~2e-3.
  - Layer-3 bias and the pad mask are injected into PSUM by one rank-2
    matmul ([b3; -BIG] x [ones; is_pad]) over the full tile, so
    p3' = W3 h2 + b3 - BIG*is_pad sits fully biased in PSUM.  BIG is only
    ~v3max+8: pads just need a negative pre-activation.
  - The post-relu segment sum is decomposed as
        sum relu(z) = 0.5*(sum z + sum |z|):
    sum z is linear: sum_slot z3 = W3 @ s2 + b3*width - BIG*npad, where
    s2 = per-slot sums of h2 accumulated for free by the Scalar engine's
    activation accumulator during the relu2 evacuation (per-slot windows).
    One epilogue matmul (moving = s2, 256 rows) plus a rank-2 correction
    matmul ([b3; -BIG] x [width; npad]) produce all 256 slot sums.
    sum |z| is one whole-tile 3D reduce_sum(apply_absolute_value) on DVE
    straight from PSUM; the segment max is the twin reduce_max (pre-relu,
    rectified once in the epilogue).  No relu3 pass, no h3 tile at all.
  - Engine balance: relu1 (mostly) and relu2 evacuations on the Scalar
    engine, both p3 reduces on the Vector engine.
  - Tiles are processed in pairs with matmuls grouped per layer, so each
    PE weight load covers two 512-column matmuls.
  - Final projection out = [sum|max|mean] @ Wo'.T + bo' runs per core on its
    own 256 segments; the 0.5 factors are folded into Wsum/Wmean on the
    host, and mean is handled by row-scaling the mean-block product by
    1/count.
"""

import os
import sys

import numpy as np

if "/opt/trn_rl_repo" not in sys.path:
    sys.path.insert(0, "/opt/trn_rl_repo")

import ml_dtypes

import concourse.bacc as bacc
import concourse.mybir as mybir
import concourse.tile as tile
from concourse import bass_utils

EPS = 1e-5
NSEG = 2048
NCORES = 8
C = 128
H = 128
O = 64
S = NSEG // NCORES  # segment slots per core (256)
MAX_TILE = 512  # PSUM bank / moving-operand limit

BF16 = ml_dtypes.bfloat16

# Per-tile engine assignment pattern for the relu1 evacuation
# ("a" = Scalar/ACT, "d" = Vector/DVE).  Tuned from trace engine-busy%.
R1_PAT = "aaad"

_compiled_cache = {}


def _fold_bn(W, b, g, be, m, v):
    a = g / np.sqrt(v + EPS)
    Wp = W * a[:, None]
    bp = (b - m) * a + be
    return Wp.astype(np.float32), bp.astype(np.float32)


def _plan_tiles(slot_w):
    """Greedy-pack slots (widths descending) into tiles of <=MAX_TILE cols.

    Returns list of (slot_start, n_slots, padded_width, col_start) and the
    total padded column count.
    """
    tiles = []
    col = 0
    k = 0
    n = len(slot_w)
    while k < n:
        wt = (int(slot_w[k]) + 1) & ~1  # keep matmul widths even
        assert 0 < wt <= MAX_TILE, f"slot width {wt} unsupported"
        d = min(MAX_TILE // wt, n - k)
        tiles.append((k, d, wt, col))
        col += d * wt
        k += d
    return tiles, col


def _build_program(tiles, cols):
    """Emit the Bass/Tile program shared by all 8 cores."""
    nc = bacc.Bacc(
        "TRN2",
        target_bir_lowering=False,
        debug=False,
        num_devices=NCORES,
    )
    f32 = mybir.dt.float32
    bf16 = mybir.dt.bfloat16

    xT = nc.dram_tensor("xT", [C, cols], bf16, kind="ExternalInput").ap()
    # aux2 row 0 = ones (bias carrier), row 1 = is_pad
    aux2 = nc.dram_tensor("aux2", [2, cols], bf16, kind="ExternalInput").ap()
    w1 = nc.dram_tensor("w1", [C, H], bf16, kind="ExternalInput").ap()
    w2 = nc.dram_tensor("w2", [H, H], bf16, kind="ExternalInput").ap()
    w3 = nc.dram_tensor("w3", [H, H], bf16, kind="ExternalInput").ap()
    b1 = nc.dram_tensor("b1", [H, 1], f32, kind="ExternalInput").ap()
    b2 = nc.dram_tensor("b2", [H, 1], f32, kind="ExternalInput").ap()
    # mb row 0 = b3, row 1 = -BIG  (stationary for mask and endfix matmuls)
    mb = nc.dram_tensor("mb", [2, H], bf16, kind="ExternalInput").ap()
    # wnp row 0 = slot padded width, row 1 = npad  (endfix moving operand)
    wnp = nc.dram_tensor("wnp", [2, S], bf16, kind="ExternalInput").ap()
    wsum = nc.dram_tensor("wsum", [H, O], f32, kind="ExternalInput").ap()
    wmax = nc.dram_tensor("wmax", [H, O], f32, kind="ExternalInput").ap()
    wmean = nc.dram_tensor("wmean", [H, O], f32, kind="ExternalInput").ap()
    bo = nc.dram_tensor("bo", [1, O], f32, kind="ExternalInput").ap()
    # column ch holds the reciprocals for segment chunk ch (128 slots each)
    recip = nc.dram_tensor("recip", [H, S // H], f32, kind="ExternalInput").ap()
    out = nc.dram_tensor("out", [S, O], f32, kind="ExternalOutput").ap()

    relu = mybir.ActivationFunctionType.Relu
    add = mybir.AluOpType.add
    amax = mybir.AluOpType.max

    with tile.TileContext(nc) as tc:
        with (
            tc.tile_pool(name="const", bufs=1) as cpool,
            tc.tile_pool(name="xin", bufs=8) as xpool,
            tc.tile_pool(name="auxin", bufs=8) as apool,
            tc.tile_pool(name="h1", bufs=6) as h1pool,
            tc.tile_pool(name="h2", bufs=6) as h2pool,
            tc.tile_pool(name="acc", bufs=1) as accpool,
            tc.tile_pool(name="ps1", bufs=3, space="PSUM") as ps1,
            tc.tile_pool(name="ps2", bufs=2, space="PSUM") as ps2,
            tc.tile_pool(name="ps3", bufs=2, space="PSUM") as ps3,
            tc.tile_pool(name="psS", bufs=1, space="PSUM") as psS,
        ):
            w1s = cpool.tile([C, H], bf16, tag="w1")
            w2s = cpool.tile([H, H], bf16, tag="w2")
            w3s = cpool.tile([H, H], bf16, tag="w3")
            b1s = cpool.tile([H, 1], f32, tag="b1")
            b2s = cpool.tile([H, 1], f32, tag="b2")
            mbs = cpool.tile([2, H], bf16, tag="mb")
            wnps = cpool.tile([2, S], bf16, tag="wnp")
            wsums = cpool.tile([H, O], f32, tag="wsum")
            wmaxs = cpool.tile([H, O], f32, tag="wmax")
            wmeans = cpool.tile([H, O], f32, tag="wmean")
            bos = cpool.tile([1, O], f32, tag="bo")
            recs = cpool.tile([H, S // H], f32, tag="recip")
            ones = cpool.tile([1, H], f32, tag="ones")

            nc.sync.dma_start(w1s[:], w1)
            nc.sync.dma_start(w2s[:], w2)
            nc.sync.dma_start(w3s[:], w3)
            nc.sync.dma_start(b1s[:], b1)
            nc.sync.dma_start(b2s[:], b2)
            nc.sync.dma_start(mbs[:], mb)
            nc.sync.dma_start(wnps[:], wnp)
            nc.sync.dma_start(wsums[:], wsum)
            nc.sync.dma_start(wmaxs[:], wmax)
            nc.sync.dma_start(wmeans[:], wmean)
            nc.sync.dma_start(bos[:], bo)
            nc.sync.dma_start(recs[:], recip)
            nc.vector.memset(ones[:], 1.0)

            # Persistent per-slot partials: pre-relu abs-sums and maxes, and
            # the per-slot sums of h2 (accumulated by the Scalar engine's
            # activation accumulator during the relu2 evacuation).
            absP = accpool.tile([H, S], f32, tag="absP")
            maxP = accpool.tile([H, S], f32, tag="maxP")
            s2 = accpool.tile([H, S], f32, tag="s2")

            def relu_evac(eng, dst, src, bias):
                if eng == "a":
                    nc.scalar.activation(dst, src, relu, bias=bias)
                else:
                    nc.vector.tensor_scalar(
                        dst, src, bias, 0.0, op0=add, op1=amax
                    )

            for pi in range(0, len(tiles), 2):
                pair = [
                    (ti, tiles[ti])
                    for ti in range(pi, min(pi + 2, len(tiles)))
                ]
                st = {}
                for ti, (k0, d, wt, col0) in pair:
                    tcols = d * wt
                    xt = xpool.tile([C, MAX_TILE], bf16, tag="xt")
                    at = apool.tile([2, MAX_TILE], bf16, tag="at")
                    nc.sync.dma_start(xt[:, :tcols], xT[:, col0 : col0 + tcols])
                    nc.sync.dma_start(at[:, :tcols], aux2[:, col0 : col0 + tcols])
                    st[ti] = dict(xt=xt, at=at, tcols=tcols)

                for ti, (k0, d, wt, col0) in pair:
                    tcols = st[ti]["tcols"]
                    p1 = ps1.tile([H, MAX_TILE], f32, tag="p1")
                    nc.tensor.matmul(
                        p1[:, :tcols], w1s[:], st[ti]["xt"][:, :tcols]
                    )
                    st[ti]["p1"] = p1
                for ti, (k0, d, wt, col0) in pair:
                    tcols = st[ti]["tcols"]
                    h1 = h1pool.tile([H, MAX_TILE], bf16, tag="h1")
                    relu_evac(
                        R1_PAT[ti % len(R1_PAT)],
                        h1[:, :tcols], st[ti]["p1"][:, :tcols], b1s[:],
                    )
                    st[ti]["h1"] = h1

                for ti, (k0, d, wt, col0) in pair:
                    tcols = st[ti]["tcols"]
                    p2 = ps2.tile([H, MAX_TILE], f32, tag="p2")
                    nc.tensor.matmul(
                        p2[:, :tcols], w2s[:], st[ti]["h1"][:, :tcols]
                    )
                    st[ti]["p2"] = p2
                for ti, (k0, d, wt, col0) in pair:
                    p2 = st[ti]["p2"]
                    h2 = h2pool.tile([H, MAX_TILE], bf16, tag="h2")
                    # Fused relu2 + per-slot h2 sums: the Scalar engine's
                    # accumulator sums the post-activation output along the
                    # free axis (one accum_out write per slot window).
                    for j in range(d):
                        wl = slice(j * wt, (j + 1) * wt)
                        nc.scalar.activation(
                            h2[:, wl], p2[:, wl], relu, bias=b2s[:],
                            accum_out=s2[:, k0 + j : k0 + j + 1],
                        )
                    st[ti]["h2"] = h2

                for ti, (k0, d, wt, col0) in pair:
                    tcols = st[ti]["tcols"]
                    p3 = ps3.tile([H, MAX_TILE], f32, tag="p3")
                    nc.tensor.matmul(
                        p3[:, :tcols], w3s[:], st[ti]["h2"][:, :tcols],
                        start=True, stop=False,
                    )
                    st[ti]["p3"] = p3
                for ti, (k0, d, wt, col0) in pair:
                    tcols = st[ti]["tcols"]
                    nc.tensor.matmul(
                        st[ti]["p3"][:, :tcols],
                        mbs[:],
                        st[ti]["at"][:, :tcols],
                        start=False,
                        stop=True,
                    )

                for ti, (k0, d, wt, col0) in pair:
                    tcols = st[ti]["tcols"]
                    p3v = st[ti]["p3"][:, :tcols].rearrange(
                        "p (d w) -> p d w", d=d
                    )
                    nc.vector.reduce_sum(
                        absP[:, k0 : k0 + d], p3v,
                        axis=mybir.AxisListType.X,
                        apply_absolute_value=True,
                    )
                    nc.vector.reduce_max(
                        maxP[:, k0 : k0 + d], p3v, axis=mybir.AxisListType.X
                    )

            # ---- epilogue ----
            # sumZ[:, k] = W3 @ s2[:, k] + b3*width_k - BIG*npad_k, then
            # sum_r = 0.5*(sumZ + absP)  (0.5 folded into wsum/wmean on host)
            # out[k, :] = sum_r @ Wsum + relu(max_k) @ Wmax
            #             + (sum_r * recip_k) @ Wmean + bo
            s2b = accpool.tile([H, S], bf16, tag="s2b")
            nc.vector.tensor_scalar_max(s2b[:], s2[:], 0.0)
            sumZ = psS.tile([H, S], f32, tag="sumZ")
            nc.tensor.matmul(sumZ[:], mbs[:], wnps[:], start=True, stop=False)
            nc.tensor.matmul(sumZ[:], w3s[:], s2b[:], start=False, stop=True)
            sumS = accpool.tile([H, S], f32, tag="sumS")
            nc.scalar.copy(sumS[:], sumZ[:])
            maxR = accpool.tile([H, S], f32, tag="maxR")
            nc.scalar.activation(maxR[:], maxP[:], relu, bias=0.0)

            for ch in range(S // H):  # 2 chunks of 128 segments
                sl = slice(ch * H, (ch + 1) * H)
                pot = ps3.tile([H, MAX_TILE], f32, tag="p3")
                po = pot[:, :O]
                nc.tensor.matmul(po, sumS[:, sl], wsums[:], start=True, stop=False)
                nc.tensor.matmul(po, absP[:, sl], wsums[:], start=False, stop=False)
                nc.tensor.matmul(po, maxR[:, sl], wmaxs[:], start=False, stop=False)
                nc.tensor.matmul(po, ones[:], bos[:], start=False, stop=True)

                pmt = ps3.tile([H, MAX_TILE], f32, tag="p3")
                pm = pmt[:, :O]
                nc.tensor.matmul(pm, sumS[:, sl], wmeans[:], start=True, stop=False)
                nc.tensor.matmul(pm, absP[:, sl], wmeans[:], start=False, stop=True)

                om = h1pool.tile([H, O], f32, tag="om")
                nc.vector.tensor_scalar_mul(om[:], pm, recs[:, ch : ch + 1])
                ot = h2pool.tile([H, O], f32, tag="ot")
                nc.vector.tensor_tensor(ot[:], po, om[:], op=mybir.AluOpType.add)
                nc.sync.dma_start(out[sl, :], ot[:])

    nc.compile()
    return nc


def kernel(**inputs):
    x = np.ascontiguousarray(np.asarray(inputs["x"], dtype=np.float32))
    batch = np.asarray(inputs["batch"]).astype(np.int64)

    # ---- fold BN into the linears ----
    W1p, b1p = _fold_bn(
        np.asarray(inputs["W1"]), np.asarray(inputs["b1"]),
        np.asarray(inputs["g1"]), np.asarray(inputs["be1"]),
        np.asarray(inputs["m1"]), np.asarray(inputs["v1"]),
    )
    W2p, b2p = _fold_bn(
        np.asarray(inputs["W2"]), np.asarray(inputs["b2"]),
        np.asarray(inputs["g2"]), np.asarray(inputs["be2"]),
        np.asarray(inputs["m2"]), np.asarray(inputs["v2"]),
    )
    W3p, b3p = _fold_bn(
        np.asarray(inputs["W3"]), np.asarray(inputs["b3"]),
        np.asarray(inputs["g3"]), np.asarray(inputs["be3"]),
        np.asarray(inputs["m3"]), np.asarray(inputs["v3"]),
    )
    Wop, bop = _fold_bn(
        np.asarray(inputs["Wo"]), np.asarray(inputs["bo"]),
        np.asarray(inputs["go"]), np.asarray(inputs["beo"]),
        np.asarray(inputs["mo"]), np.asarray(inputs["vo"]),
    )

    # Pad columns are zero in x, so h2_pad is a known constant; BIG only
    # needs to push the padded layer-3 pre-activation below zero.
    h1_pad = np.maximum(b1p, 0.0)
    h2_pad = np.maximum(W2p @ h1_pad + b2p, 0.0)
    v3 = W3p @ h2_pad + b3p
    BIG = float(np.float32(BF16(max(0.0, float(v3.max())) + 8.0)))

    # ---- whole-segment sharding by sorted-width round-robin rank ----
    counts = np.bincount(batch, minlength=NSEG).astype(np.int64)
    assert np.all(batch[:-1] <= batch[1:]), "batch must be sorted"
    order = np.argsort(-counts, kind="stable")  # segment ids, width desc
    slot_w = np.maximum(counts[order[::NCORES][:S]], 1)  # width of rank 8k
    tiles, cols = _plan_tiles(slot_w)

    key = (cols, float(BIG), tuple(slot_w.tolist()))
    if key not in _compiled_cache:
        _compiled_cache[key] = _build_program(tiles, cols)
    nc = _compiled_cache[key]

    # column start and padded width of each slot
    slot_col = np.zeros(S, dtype=np.int64)
    slot_wt = np.zeros(S, dtype=np.int64)
    for k0, d, wt, col0 in tiles:
        for j in range(d):
            slot_col[k0 + j] = col0 + j * wt
            slot_wt[k0 + j] = wt

    starts = np.searchsorted(batch, np.arange(NSEG), side="left")
    ends = np.searchsorted(batch, np.arange(NSEG), side="right")

    in_maps = []
    for c in range(NCORES):
        segs = order[np.arange(S) * NCORES + c]  # this core's segment ids
        src = np.full(cols, -1, dtype=np.int64)
        for k in range(S):
            s = segs[k]
            cnt = int(counts[s])
            if cnt:
                src[slot_col[k] : slot_col[k] + cnt] = np.arange(
                    starts[s], ends[s]
                )
        real = src >= 0
        xTc = np.zeros((C, cols), dtype=BF16)
        xTc[:, real] = x[src[real]].astype(BF16).T
        aux2c = np.zeros((2, cols), dtype=BF16)
        aux2c[0, :] = 1.0
        aux2c[1, ~real] = 1.0
        npad = (slot_wt - np.minimum(counts[segs], slot_wt)).astype(np.float64)
        wnpc = np.stack(
            [slot_wt.astype(np.float64), npad]
        ).astype(BF16)
        recipc = (1.0 / np.maximum(counts[segs], 1.0)).astype(np.float32)
        mbc = np.stack(
            [b3p.astype(np.float64), np.full(H, -BIG, np.float64)]
        ).astype(BF16)
        in_maps.append(
            dict(
                xT=xTc,
                aux2=aux2c,
                w1=np.ascontiguousarray(W1p.T.astype(BF16)),
                w2=np.ascontiguousarray(W2p.T.astype(BF16)),
                w3=np.ascontiguousarray(W3p.T.astype(BF16)),
                b1=np.ascontiguousarray(b1p[:, None]),
                b2=np.ascontiguousarray(b2p[:, None]),
                mb=np.ascontiguousarray(mbc),
                wnp=np.ascontiguousarray(wnpc),
                wsum=np.ascontiguousarray(0.5 * Wop[:, 0:H].T),
                wmax=np.ascontiguousarray(Wop[:, H : 2 * H].T),
                wmean=np.ascontiguousarray(0.5 * Wop[:, 2 * H : 3 * H].T),
                bo=np.ascontiguousarray(bop[None, :]),
                recip=np.ascontiguousarray(recipc.reshape(S // H, H).T),
            )
        )

    ncores_run = int(os.environ.get("KERNEL_NCORES", str(NCORES)))
    res = bass_utils.run_bass_kernel_spmd(
        nc,
        in_maps[:ncores_run],
        core_ids=list(range(ncores_run)),
        trace=bool(int(os.environ.get("KERNEL_TRACE", "0"))),
        tmpdir=os.environ.get("KERNEL_TRACE_DIR") or None,
    )
    kernel.last_results = res

    out_full = np.zeros((NSEG, O), dtype=np.float32)
    ranks = np.arange(S)
    for c in range(ncores_run):
        out_full[order[ranks * NCORES + c]] = res.results[c]["out"]
    return out_full
